# revision 51
# baseline (speedup 1.0000x reference)
"""Trainium2 Bass kernel for the DocRED-style segment_reduce model.

Sharding: 8 cores, data-parallel: core c -> (doc = c//2, pair-half = c%2).
Each core independently computes logits for its 256 pairs. No collectives.
All segment reductions / gathers are lowered to one-hot matmuls whose
one-hot matrices are built on the host from the integer inputs and passed
as per-core input tensors (the SPMD program itself is index-agnostic).
"""

import os

import numpy as np

import concourse.bacc as bacc
import concourse.bass as bass
import concourse.mybir as mybir
import concourse.tile as tile
from concourse.bass_utils import run_bass_kernel_spmd

B, M, H = 4, 128, 1024
NH, L = 16, 1024
E, R = 64, 512
EMB, BS, NCL = 768, 64, 97
K12 = EMB // BS  # 12 blocks
NCORES = 8
RPC = R // 2  # pairs per core

F32 = mybir.dt.float32
F32R = mybir.dt.float32r
BF16 = mybir.dt.bfloat16

# matmul/compute dtype mode: "f32" | "f32r" | "bf16"
MM_MODE = os.environ.get("DOCRED_MM_MODE", "bf16")


def _fdt():
    return BF16 if MM_MODE == "bf16" else F32


def _np_fdt():
    import ml_dtypes

    return np.dtype(ml_dtypes.bfloat16) if MM_MODE == "bf16" else np.float32


class _Builder:
    def __init__(self, mm_mode: str):
        self.mm_mode = mm_mode
        self.fdt = {"f32": F32, "f32r": F32R, "bf16": BF16}[mm_mode]
        nc = bacc.Bacc("TRN2", target_bir_lowering=False, debug=False)
        self.nc = nc
        fdt = self.fdt
        # ---- DRAM tensors (per-core inputs) ----
        d = {}
        d["ent"] = nc.dram_tensor("ent", [M, H], F32, kind="ExternalInput")
        d["attn"] = nc.dram_tensor("attn", [M, NH * L], fdt, kind="ExternalInput")
        d["seq"] = nc.dram_tensor("seq", [128, 8 * (L + 1)], fdt, kind="ExternalInput")
        d["ssum"] = nc.dram_tensor("ssum", [M, E], fdt, kind="ExternalInput")
        d["ohxy2"] = nc.dram_tensor("ohxy2", [M, 2 * RPC], fdt,
                                    kind="ExternalInput")
        d["eadd"] = nc.dram_tensor("eadd", [E, 1], F32, kind="ExternalInput")
        d["ohx"] = nc.dram_tensor("ohx", [E, RPC], fdt, kind="ExternalInput")
        d["ohy"] = nc.dram_tensor("ohy", [E, RPC], fdt, kind="ExternalInput")
        d["wh"] = nc.dram_tensor("wh", [128, 16 * EMB], fdt, kind="ExternalInput")
        d["wt"] = nc.dram_tensor("wt", [128, 16 * EMB], fdt, kind="ExternalInput")
        d["bh"] = nc.dram_tensor("bh", [128, EMB // 128], F32, kind="ExternalInput")
        d["bt"] = nc.dram_tensor("bt", [128, EMB // 128], F32, kind="ExternalInput")
        d["wb"] = nc.dram_tensor("wb", [128, 384 * NCL], fdt, kind="ExternalInput")
        d["bbc"] = nc.dram_tensor("bbc", [NCL, 1], F32, kind="ExternalInput")
        d["ident"] = nc.dram_tensor("ident", [128, 128], fdt, kind="ExternalInput")
        d["repm"] = nc.dram_tensor("repm", [E, 32 * 128], fdt, kind="ExternalInput")
        d["lt"] = nc.dram_tensor("lt", [NCL, RPC], F32, kind="ExternalOutput")
        self.d = d
        with tile.TileContext(nc) as tc:
            self.build(tc)
        nc.compile()

    def mm(self, out, lhsT, rhs, **kw):
        return self.nc.tensor.matmul(out, lhsT, rhs, **kw)

    def tp(self, out, in_, ident, **kw):
        return self.nc.tensor.matmul(out, in_, ident, is_transpose=True, **kw)

    def build(self, tc):
        nc = self.nc
        d = self.d
        fdt = self.fdt
        AF = mybir.ActivationFunctionType

        with (
            tc.tile_pool(name="pin", bufs=1) as pin,
            tc.tile_pool(name="mid", bufs=1) as mid,
            tc.tile_pool(name="late", bufs=1) as late,
            tc.tile_pool(name="gx", bufs=3) as gxpool,
            tc.tile_pool(name="dramp", bufs=1, space="DRAM") as dramp,
        ):
            # ---------- load small persistent tensors ----------
            ssum = pin.tile([M, E], fdt)
            ohxy2 = pin.tile([M, 2 * RPC], fdt)
            eadd = pin.tile([E, 1], F32)
            ohx = pin.tile([E, RPC], fdt)
            ohy = pin.tile([E, RPC], fdt)
            ident = pin.tile([128, 128], fdt)
            bh = pin.tile([128, EMB // 128], F32)
            bt = pin.tile([128, EMB // 128], F32)
            bbc = pin.tile([NCL, 1], F32)
            repm = pin.tile([E, 32, 128], fdt)
            for t, key in [
                (ident, "ident"), (ohxy2, "ohxy2"), (ssum, "ssum"),
                (eadd, "eadd"), (ohx, "ohx"), (ohy, "ohy"),
                (bh, "bh"), (bt, "bt"), (bbc, "bbc"),
            ]:
                nc.sync.dma_start(t[:], d[key].ap())
            nc.sync.dma_start(repm[:], d["repm"].ap()
                              .rearrange("p (a b) -> p a b", a=32))

            # ---------- P1: exp + segment-sum + log ----------
            psA_cm = tc.tile_pool(name="psA", bufs=2, space="PSUM")
            ps_big = ps_sm = psA_cm.__enter__()
            ent = mid.tile([M, H], F32)
            nc.sync.dma_start(ent[:], d["ent"].ap())
            if self.mm_mode != "f32":
                pexp = mid.tile([M, H], fdt, name="pexp")
            else:
                pexp = ent
            nc.scalar.activation(pexp[:], ent[:], AF.Exp)
            ps_ent = ps_big.tile([E, H], F32, tag="big")
            for nh in range(2):
                self.mm(ps_ent[:, nh * 512:(nh + 1) * 512], ssum[:],
                        pexp[:, nh * 512:(nh + 1) * 512])
            ent_sb = mid.tile([E, H], fdt)
            nc.scalar.activation(ent_sb[:], ps_ent[:], AF.Ln, bias=eadd[:])

            # ---------- P2: EW = ent_sb @ W[0:1024] (gather commutes with
            # the linear extractor half: gather(ent)@W == gather(ent@W)) ----
            entT = mid.tile([128, 8, E], fdt, name="entT")
            for hc in range(8):
                ps_t = ps_sm.tile([128, E], fdt, tag="sm", name="ps_t")
                self.tp(ps_t[:], ent_sb[:, hc * 128:(hc + 1) * 128],
                        ident[0:E, 0:E])
                nc.scalar.copy(entT[:, hc, :], ps_t[:])
            wpin_cm = tc.tile_pool(name="wpin", bufs=1)
            wpin = wpin_cm.__enter__()
            wh_sb = wpin.tile([128, 16, EMB], fdt, name="wh_sb")
            nc.sync.dma_start(
                wh_sb[:], d["wh"].ap().rearrange("p (a b) -> p a b", a=16))
            wt_sb = wpin.tile([128, 16, EMB], fdt, name="wt_sb")
            nc.sync.dma_start(
                wt_sb[:], d["wt"].ap().rearrange("p (a b) -> p a b", a=16))
            EWh = mid.tile([E, EMB], fdt, name="EWh")
            EWt = mid.tile([E, EMB], fdt, name="EWt")
            for w, ew in ((wh_sb, EWh), (wt_sb, EWt)):
                ps_ew = ps_big.tile([E, EMB], F32, tag="big", name="ps_ew")
                for hc in range(8):
                    for lo, hi in ((0, 512), (512, 768)):
                        self.mm(ps_ew[:, lo:hi], entT[:, hc, :],
                                w[:, hc, lo:hi],
                                start=(hc == 0), stop=(hc == 7))
                nc.scalar.copy(ew[:], ps_ew[:])

            psA_cm.__exit__(None, None, None)

            # ---------- P3: C = sum_h gather_x(attn_h) * gather_y(attn_h) ---
            # lc-outer, all heads resident, tree-reduction of products
            psC_cm = tc.tile_pool(name="psC", bufs=4, space="PSUM")
            ps_big = psC_cm.__enter__()
            CTmm = mid.tile([128, 8, RPC], fdt, name="CTmm")
            with tc.tile_pool(name="ahpool", bufs=1) as ahpool:
                attn = ahpool.tile([M, NH, L], fdt)
                av = d["attn"].ap().rearrange("p (h l) -> p h l", h=NH)
                for q in range(4):
                    nc.sync.dma_start(attn[:, 4 * q:4 * (q + 1), :],
                                      av[:, 4 * q:4 * (q + 1), :])
                for lc in range(8):
                    tmps = []
                    for g in range(NH // 2):
                        # one N=512 matmul gathers x and y for each head
                        ps_xy = ps_big.tile([128, 2, 2, RPC], F32, tag="c4",
                                            name="ps_xy")
                        for hh in range(2):
                            a_sl = attn[:, 2 * g + hh, lc * 128:(lc + 1) * 128]
                            self.mm(ps_xy[:, hh], a_sl, ohxy2[:])
                        gxs2 = gxpool.tile([128, 2, RPC], fdt, tag="gxs",
                                           bufs=3)
                        nc.scalar.copy(gxs2[:], ps_xy[:, :, 0, :])
                        tg = gxpool.tile([128, 2, RPC], fdt, tag=f"ct{g}",
                                         bufs=2, name=f"tg{g}")
                        nc.vector.tensor_mul(tg[:], gxs2[:], ps_xy[:, :, 1, :])
                        tmps.append(tg)
                    # tree reduce: 8 tiles [128, 2, RPC] -> CTmm[:, lc, :]
                    for lvl in (4, 2, 1):
                        for j in range(lvl):
                            nc.vector.tensor_add(tmps[j][:], tmps[j][:],
                                                 tmps[j + lvl][:])
                    nc.vector.tensor_add(CTmm[:, lc, :], tmps[0][:, 0, :],
                                         tmps[0][:, 1, :])

                psC_cm.__exit__(None, None, None)
                # ------ P4: rel = (C @ [seq|1]), normalize, transpose ----
                psR_cm = tc.tile_pool(name="psR", bufs=2, space="PSUM")
                ps_big = ps_sm = psR_cm.__enter__()
                relT = mid.tile([128, 8, RPC], fdt)
                ps_rel = [ps_big.tile([128, L], F32, tag="big",
                                      name=f"ps_rel{i}") for i in range(2)]
                ps_s8 = ps_sm.tile([128, 2, 8], F32, tag="ss", name="ps_s8",
                                   bufs=1)
                seq_view = d["seq"].ap().rearrange("p (a b) -> p a b", a=8)
                for lc in range(8):
                    sq = ahpool.tile([128, L + 1], fdt, tag="sq", bufs=3)
                    nc.sync.dma_start(sq[:], seq_view[:, lc, :])
                    st, sp = lc == 0, lc == 7
                    for rc in range(2):
                        lhsT = CTmm[:, lc, rc * 128:(rc + 1) * 128]
                        self.mm(ps_rel[rc][:, 0:512], lhsT, sq[:, 0:512],
                                start=st, stop=sp)
                        self.mm(ps_rel[rc][:, 512:1024], lhsT, sq[:, 512:1024],
                                start=st, stop=sp)
                        sl = (lhsT, sq[:, 1024:1025])
                        if self.mm_mode == "f32r":
                            sl = (sl[0].bitcast(F32), sl[1].bitcast(F32))
                        self.mm(ps_s8[:, rc, lc:lc + 1], sl[0], sl[1],
                                start=True, stop=True)
                for rc in range(2):
                    tdenom = gxpool.tile([128, 1], F32, tag="tden")
                    nc.vector.tensor_reduce(tdenom[:], ps_s8[:, rc, :],
                                            axis=mybir.AxisListType.X,
                                            op=mybir.AluOpType.add)
                    nc.scalar.activation(tdenom[:], tdenom[:], AF.Copy,
                                         bias=16e-5, scale=1.0)
                    frec = gxpool.tile([128, 1], F32, tag="frec")
                    nc.vector.reciprocal(frec[:], tdenom[:])
                    rel_sc = mid.tile([128, L], fdt, tag="rel_sc",
                                      name="rel_sc")
                    nc.vector.tensor_scalar_mul(rel_sc[:], ps_rel[rc][:],
                                                frec[:])
                    for dc in range(8):
                        ps_t = ps_sm.tile([128, 128], fdt, tag="sm")
                        self.tp(ps_t[:], rel_sc[:, dc * 128:(dc + 1) * 128],
                                ident[:])
                        nc.scalar.copy(relT[:, dc, rc * 128:(rc + 1) * 128],
                                       ps_t[:])

            psR_cm.__exit__(None, None, None)
            # ---------- P5: extractors -> hsEt/tsEt [emb, n], per-ec -------
            # h/t interleaved per emb-chunk and staged to DRAM immediately so
            # classifier block k can start once chunk k//2 is staged.
            psE_cm = tc.tile_pool(name="psE", bufs=3, space="PSUM")
            ps_sm = psE_cm.__enter__()
            hsEt = late.tile([128, 6, RPC], fdt)
            tsEt = late.tile([128, 6, RPC], fdt)
            hsDs = [dramp.tile([128, RPC], fdt, name=f"hsD{e}")
                    for e in range(6)]
            tsDs = [dramp.tile([128, RPC], fdt, name=f"tsD{e}")
                    for e in range(6)]
            if True:
                for ec in range(6):
                    for (w, bvec, ew, oh, dst, dv) in (
                        (wh_sb, bh, EWh, ohx, hsEt, hsDs[ec]),
                        (wt_sb, bt, EWt, ohy, tsEt, tsDs[ec]),
                    ):
                        ps_e = ps_sm.tile([128, RPC], F32, tag="sm",
                                          name="ps_e")
                        self.mm(ps_e[:], ew[:, ec * 128:(ec + 1) * 128], oh[:],
                                start=True, stop=False)
                        for kc in range(8, 16):
                            self.mm(ps_e[:], w[:, kc, ec * 128:(ec + 1) * 128],
                                    relT[:, kc % 8, :],
                                    start=False, stop=(kc == 15))
                        nc.scalar.activation(dst[:, ec, :], ps_e[:], AF.Tanh,
                                             bias=bvec[:, ec:ec + 1])
                        nc.sync.dma_start(dv[:], dst[:, ec, :])
            psE_cm.__exit__(None, None, None)
            wpin_cm.__exit__(None, None, None)

            # ---------- P6: block bilinear + classifier ----------
            with (
                tc.tile_pool(name="blph", bufs=3) as blph,
                tc.tile_pool(name="ps_lt", bufs=1, space="PSUM") as ps_lt,
            ):
                pslt = ps_lt.tile([NCL, RPC], F32)
                for k in range(K12):
                    wb = blph.tile([128, 32 * NCL], fdt, tag="wb")
                    nc.sync.dma_start(
                        wb[:],
                        d["wb"].ap()[:, k * 32 * NCL:(k + 1) * 32 * NCL])
                    # b1rep[p, c, n] = hsEt_row(64k + 32*(p//64) + c)[n]
                    # (wb host layout matches: i = c + 32*(p//64), j = p%64)
                    kk = 64 * (k % 2)
                    # b2t[p, n] = tsEt_row(64k + p%64)[n]
                    b2t = blph.tile([128, RPC], fdt, tag="b2t")
                    for h0 in (0, 1):
                        nc.scalar.dma_start(b2t[64 * h0:64 * (h0 + 1)],
                                            tsDs[k // 2][kk:kk + 64, :])
                    blT = blph.tile([128, 32, RPC], fdt, tag="blT")
                    if k % 2 == 0:
                        # PE-route: replicate hsEt rows via one-hot matmuls,
                        # ACT-copy out of PSUM, multiply on DVE
                        hsE64 = hsEt[kk:kk + 64, k // 2, :]
                        for cq in range(8):
                            psR = ps_lt.tile([128, 4, RPC], F32, tag="rep",
                                             bufs=3, name="psR")
                            for i4 in range(4):
                                self.mm(psR[:, i4, :],
                                        repm[:, cq * 4 + i4, :], hsE64)
                            b1s = blph.tile([128, 4, RPC], fdt, tag="b1s",
                                            bufs=3, name="b1s")
                            nc.scalar.copy(b1s[:], psR[:])
                            b2b = b2t[:].unsqueeze(1).broadcast_to(
                                [128, 4, RPC])
                            nc.vector.tensor_mul(
                                blT[:, cq * 4:(cq + 1) * 4, :], b1s[:], b2b)
                    else:
                        b1rep = blph.tile([128, 32, RPC], fdt, tag="b1rep")
                        for h0 in (0, 1):
                            srcap = hsDs[k // 2] \
                                [kk + 32 * h0:kk + 32 * (h0 + 1), :] \
                                .unsqueeze(0).broadcast_to([64, 32, RPC])
                            nc.scalar.dma_start(b1rep[64 * h0:64 * (h0 + 1)],
                                                srcap)
                        for ch in range(4):
                            b2b = b2t[:].unsqueeze(1).broadcast_to(
                                [128, 8, RPC])
                            nc.vector.tensor_mul(
                                blT[:, 8 * ch:8 * (ch + 1), :],
                                b1rep[:, 8 * ch:8 * (ch + 1), :], b2b)
                    for c in range(32):
                        cg = k * 32 + c
                        self.mm(pslt[:], wb[:, c * NCL:(c + 1) * NCL],
                                blT[:, c, :],
                                start=(cg == 0), stop=(cg == 383))

                out_sb = late.tile([NCL, RPC], F32)
                nc.scalar.activation(out_sb[:], pslt[:], AF.Identity,
                                     bias=bbc[:])
                nc.sync.dma_start(d["lt"].ap(), out_sb[:])


_PROGRAM_CACHE = {}


def _get_program(mm_mode: str):
    if mm_mode not in _PROGRAM_CACHE:
        _PROGRAM_CACHE[mm_mode] = _Builder(mm_mode)
    return _PROGRAM_CACHE[mm_mode]


def _host_inputs(seq_lhs, ent_lhs, ent_to_seq_attn, entity_id_labels, hts,
                 Wh, bh, Wt, bt, Wb, bb):
    """Build the 8 per-core input maps (all host-side numpy)."""
    fdt = _np_fdt()
    seq_lhs = np.asarray(seq_lhs, np.float32)
    ent_lhs = np.asarray(ent_lhs, np.float32)
    ent_to_seq_attn = np.asarray(ent_to_seq_attn, np.float32)
    entity_id_labels = np.asarray(entity_id_labels)
    hts = np.asarray(hts)
    Wh = np.asarray(Wh, np.float32)
    Wt = np.asarray(Wt, np.float32)
    Wb = np.asarray(Wb, np.float32)
    bh = np.asarray(bh, np.float32)
    bt = np.asarray(bt, np.float32)
    bb = np.asarray(bb, np.float32)

    # device chunk (k, c) row p maps to Wb row k*4096 + i*64 + j with
    # i = c + 32*(p//64), j = p%64
    p_ = np.arange(128)
    c_ = np.arange(32)
    k_ = np.arange(K12)
    rows = (k_[:, None, None] * 4096
            + (c_[None, :, None] + 32 * (p_[None, None, :] // 64)) * 64
            + (p_[None, None, :] % 64))  # [k, c, p]
    wb_r = np.ascontiguousarray(
        Wb[rows.reshape(-1), :].reshape(K12 * 32, 128, NCL)
        .transpose(1, 0, 2).reshape(128, 384 * NCL)
    ).astype(fdt)
    wh_c = np.ascontiguousarray(
        Wh.reshape(16, 128, EMB).transpose(1, 0, 2).reshape(128, 16 * EMB)
    ).astype(fdt)
    wt_c = np.ascontiguousarray(
        Wt.reshape(16, 128, EMB).transpose(1, 0, 2).reshape(128, 16 * EMB)
    ).astype(fdt)
    bh_c = np.ascontiguousarray(bh.reshape(EMB // 128, 128).T)
    bt_c = np.ascontiguousarray(bt.reshape(EMB // 128, 128).T)
    bb_c = np.ascontiguousarray(bb.reshape(NCL, 1))
    ident = np.eye(128, dtype=np.float32).astype(fdt)
    # repm[r, c, p] = 1 iff r == c + 32*(p//64)
    repm_h = np.zeros((E, 32, 128), np.float32)
    for c in range(32):
        repm_h[c, c, 0:64] = 1.0
        repm_h[c + 32, c, 64:128] = 1.0
    repm_h = repm_h.reshape(E, 32 * 128).astype(fdt)

    in_maps = []
    for c in range(NCORES):
        doc, half = divmod(c, 2)
        sl = slice(half * RPC, (half + 1) * RPC)
        labels = entity_id_labels[doc].astype(np.int64)
        cnt = np.bincount(labels, minlength=E).astype(np.float32)
        S = np.zeros((M, E), np.float32)
        S[np.arange(M), labels] = 1.0
        smean = S / np.maximum(cnt, 1.0)[None, :]  # [M, E]
        eadd = (cnt == 0).astype(np.float32).reshape(E, 1)
        hi = hts[doc, sl, 0].astype(np.int64)
        ti = hts[doc, sl, 1].astype(np.int64)
        ohx = np.zeros((E, RPC), np.float32)
        ohx[hi, np.arange(RPC)] = 1.0
        ohy = np.zeros((E, RPC), np.float32)
        ohy[ti, np.arange(RPC)] = 1.0
        ohxy2 = np.concatenate([smean @ ohx, smean @ ohy], axis=1)  # [M, 512]
        attn = np.ascontiguousarray(
            ent_to_seq_attn[doc].transpose(1, 0, 2).reshape(M, NH * L)
        ).astype(fdt)
        seq_r = seq_lhs[doc].reshape(8, 128, L).transpose(1, 0, 2)
        seq_aug = np.concatenate(
            [seq_r, np.ones((128, 8, 1), np.float32)], axis=2
        )
        in_maps.append({
            "ent": np.ascontiguousarray(ent_lhs[doc]),
            "attn": attn,
            "seq": np.ascontiguousarray(seq_aug.reshape(128, 8 * (L + 1))).astype(fdt),
            "ssum": S.astype(fdt),
            "ohxy2": ohxy2.astype(fdt),
            "eadd": eadd,
            "ohx": ohx.astype(fdt),
            "ohy": ohy.astype(fdt),
            "wh": wh_c, "wt": wt_c, "bh": bh_c, "bt": bt_c,
            "wb": wb_r, "bbc": bb_c, "ident": ident, "repm": repm_h,
        })
    return in_maps


_LAST_RESULTS = {}


def kernel(**inputs) -> np.ndarray:
    prog = _get_program(MM_MODE)
    in_maps = _host_inputs(**inputs)
    trace = os.environ.get("DOCRED_TRACE", "0") == "1"
    res = run_bass_kernel_spmd(
        prog.nc, in_maps, core_ids=list(range(NCORES)), trace=trace,
    )
    _LAST_RESULTS["res"] = res
    out = np.empty((B * R, NCL), np.float32)
    for c in range(NCORES):
        doc, half = divmod(c, 2)
        lt = res.results[c]["lt"]  # [NCL, RPC]
        out[doc * R + half * RPC: doc * R + (half + 1) * RPC, :] = lt.T
    return out


# revision 53
# speedup vs baseline: 1.0661x; 1.0661x over previous
"""Trainium2 Bass kernel for the DocRED-style segment_reduce model.

Sharding: 8 cores, data-parallel: core c -> (doc = c//2, pair-half = c%2).
Each core independently computes logits for its 256 pairs. No collectives.
All segment reductions / gathers are lowered to one-hot matmuls whose
one-hot matrices are built on the host from the integer inputs and passed
as per-core input tensors (the SPMD program itself is index-agnostic).
"""

import os

import numpy as np

import concourse.bacc as bacc
import concourse.bass as bass
import concourse.mybir as mybir
import concourse.tile as tile
from concourse.bass_utils import run_bass_kernel_spmd

B, M, H = 4, 128, 1024
NH, L = 16, 1024
E, R = 64, 512
EMB, BS, NCL = 768, 64, 97
K12 = EMB // BS  # 12 blocks
NCORES = 8
RPC = R // 2  # pairs per core

F32 = mybir.dt.float32
F32R = mybir.dt.float32r
BF16 = mybir.dt.bfloat16

# matmul/compute dtype mode: "f32" | "f32r" | "bf16"
MM_MODE = os.environ.get("DOCRED_MM_MODE", "bf16")


def _fdt():
    return BF16 if MM_MODE == "bf16" else F32


def _np_fdt():
    import ml_dtypes

    return np.dtype(ml_dtypes.bfloat16) if MM_MODE == "bf16" else np.float32


class _Builder:
    def __init__(self, mm_mode: str):
        self.mm_mode = mm_mode
        self.fdt = {"f32": F32, "f32r": F32R, "bf16": BF16}[mm_mode]
        nc = bacc.Bacc("TRN2", target_bir_lowering=False, debug=False)
        self.nc = nc
        fdt = self.fdt
        # ---- DRAM tensors (per-core inputs) ----
        d = {}
        d["ent"] = nc.dram_tensor("ent", [M, H], F32, kind="ExternalInput")
        d["attn"] = nc.dram_tensor("attn", [M, NH * L], fdt, kind="ExternalInput")
        d["seq"] = nc.dram_tensor("seq", [128, 8 * (L + 1)], fdt, kind="ExternalInput")
        d["ssum"] = nc.dram_tensor("ssum", [M, E], fdt, kind="ExternalInput")
        d["ohxy2"] = nc.dram_tensor("ohxy2", [M, 2 * RPC], fdt,
                                    kind="ExternalInput")
        d["eadd"] = nc.dram_tensor("eadd", [E, 1], F32, kind="ExternalInput")
        d["ohx"] = nc.dram_tensor("ohx", [E, RPC], fdt, kind="ExternalInput")
        d["ohy"] = nc.dram_tensor("ohy", [E, RPC], fdt, kind="ExternalInput")
        d["wh"] = nc.dram_tensor("wh", [128, 16 * EMB], fdt, kind="ExternalInput")
        d["wt"] = nc.dram_tensor("wt", [128, 16 * EMB], fdt, kind="ExternalInput")
        d["bh"] = nc.dram_tensor("bh", [128, EMB // 128], F32, kind="ExternalInput")
        d["bt"] = nc.dram_tensor("bt", [128, EMB // 128], F32, kind="ExternalInput")
        d["wb"] = nc.dram_tensor("wb", [128, 384 * NCL], fdt, kind="ExternalInput")
        d["bbc"] = nc.dram_tensor("bbc", [NCL, 1], F32, kind="ExternalInput")
        d["ident"] = nc.dram_tensor("ident", [128, 128], fdt, kind="ExternalInput")
        d["repm"] = nc.dram_tensor("repm", [E, 32 * 128], fdt, kind="ExternalInput")
        d["lt"] = nc.dram_tensor("lt", [NCL, RPC], F32, kind="ExternalOutput")
        self.d = d
        with tile.TileContext(nc) as tc:
            self.build(tc)
        nc.compile()

    def mm(self, out, lhsT, rhs, **kw):
        return self.nc.tensor.matmul(out, lhsT, rhs, **kw)

    def tp(self, out, in_, ident, **kw):
        return self.nc.tensor.matmul(out, in_, ident, is_transpose=True, **kw)

    def build(self, tc):
        nc = self.nc
        d = self.d
        fdt = self.fdt
        AF = mybir.ActivationFunctionType

        with (
            tc.tile_pool(name="pin", bufs=1) as pin,
            tc.tile_pool(name="mid", bufs=1) as mid,
            tc.tile_pool(name="late", bufs=1) as late,
            tc.tile_pool(name="gx", bufs=3) as gxpool,
            tc.tile_pool(name="dramp", bufs=1, space="DRAM") as dramp,
        ):
            # ---------- load small persistent tensors ----------
            ssum = pin.tile([M, E], fdt)
            ohxy2 = pin.tile([M, 2 * RPC], fdt)
            eadd = pin.tile([E, 1], F32)
            ohx = pin.tile([E, RPC], fdt)
            ohy = pin.tile([E, RPC], fdt)
            ident = pin.tile([128, 128], fdt)
            bh = pin.tile([128, EMB // 128], F32)
            bt = pin.tile([128, EMB // 128], F32)
            bbc = pin.tile([NCL, 1], F32)
            repm = pin.tile([E, 32, 128], fdt)
            for t, key in [
                (ident, "ident"), (ohxy2, "ohxy2"), (ssum, "ssum"),
                (eadd, "eadd"), (ohx, "ohx"), (ohy, "ohy"),
                (bh, "bh"), (bt, "bt"), (bbc, "bbc"),
            ]:
                nc.sync.dma_start(t[:], d[key].ap())
            nc.sync.dma_start(repm[:], d["repm"].ap()
                              .rearrange("p (a b) -> p a b", a=32))

            # ---------- P1: exp + segment-sum + log ----------
            psA_cm = tc.tile_pool(name="psA", bufs=2, space="PSUM")
            ps_big = ps_sm = psA_cm.__enter__()
            ent = mid.tile([M, H], F32)
            nc.sync.dma_start(ent[:], d["ent"].ap())
            if self.mm_mode != "f32":
                pexp = mid.tile([M, H], fdt, name="pexp")
            else:
                pexp = ent
            nc.scalar.activation(pexp[:], ent[:], AF.Exp)
            ps_ent = ps_big.tile([E, H], F32, tag="big")
            for nh in range(2):
                self.mm(ps_ent[:, nh * 512:(nh + 1) * 512], ssum[:],
                        pexp[:, nh * 512:(nh + 1) * 512])
            ent_sb = mid.tile([E, H], fdt)
            nc.scalar.activation(ent_sb[:], ps_ent[:], AF.Ln, bias=eadd[:])

            wpin_cm = tc.tile_pool(name="wpin", bufs=1)
            wpin = wpin_cm.__enter__()
            wh_sb = wpin.tile([128, 16, EMB], fdt, name="wh_sb")
            wt_sb = wpin.tile([128, 16, EMB], fdt, name="wt_sb")

            psA_cm.__exit__(None, None, None)

            # ---------- P3: C = sum_h gather_x(attn_h) * gather_y(attn_h) ---
            # lc-outer, all heads resident, tree-reduction of products
            psC_cm = tc.tile_pool(name="psC", bufs=4, space="PSUM")
            ps_big = psC_cm.__enter__()
            CTmm = mid.tile([128, 8, RPC], fdt, name="CTmm")
            with tc.tile_pool(name="ahpool", bufs=1) as ahpool:
                attn = ahpool.tile([M, NH, L], fdt)
                av = d["attn"].ap().rearrange("p (h l) -> p h l", h=NH)
                for q in range(4):
                    nc.sync.dma_start(attn[:, 4 * q:4 * (q + 1), :],
                                      av[:, 4 * q:4 * (q + 1), :])
                nc.sync.dma_start(
                    wh_sb[:], d["wh"].ap().rearrange("p (a b) -> p a b", a=16))
                nc.sync.dma_start(
                    wt_sb[:], d["wt"].ap().rearrange("p (a b) -> p a b", a=16))
                for lc in range(8):
                    tmps = []
                    for g in range(NH // 2):
                        # one N=512 matmul gathers x and y for each head
                        ps_xy = ps_big.tile([128, 2, 2, RPC], F32, tag="c4",
                                            name="ps_xy")
                        for hh in range(2):
                            a_sl = attn[:, 2 * g + hh, lc * 128:(lc + 1) * 128]
                            self.mm(ps_xy[:, hh], a_sl, ohxy2[:])
                        gxs2 = gxpool.tile([128, 2, RPC], fdt, tag="gxs",
                                           bufs=3)
                        nc.scalar.copy(gxs2[:], ps_xy[:, :, 0, :])
                        tg = gxpool.tile([128, 2, RPC], fdt, tag=f"ct{g}",
                                         bufs=2, name=f"tg{g}")
                        nc.vector.tensor_mul(tg[:], gxs2[:], ps_xy[:, :, 1, :])
                        tmps.append(tg)
                    # tree reduce: 8 tiles [128, 2, RPC] -> CTmm[:, lc, :]
                    for lvl in (4, 2, 1):
                        for j in range(lvl):
                            nc.vector.tensor_add(tmps[j][:], tmps[j][:],
                                                 tmps[j + lvl][:])
                    nc.vector.tensor_add(CTmm[:, lc, :], tmps[0][:, 0, :],
                                         tmps[0][:, 1, :])

                psC_cm.__exit__(None, None, None)
                # ------ P4: rel = (C @ [seq|1]), normalize, transpose ----
                psR_cm = tc.tile_pool(name="psR", bufs=2, space="PSUM")
                ps_big = ps_sm = psR_cm.__enter__()
                relT = mid.tile([128, 8, RPC], fdt)
                ps_rel = [ps_big.tile([128, L], F32, tag="big",
                                      name=f"ps_rel{i}") for i in range(2)]
                ps_s8 = ps_sm.tile([128, 2, 8], F32, tag="ss", name="ps_s8",
                                   bufs=1)
                seq_view = d["seq"].ap().rearrange("p (a b) -> p a b", a=8)
                for lc in range(8):
                    sq = ahpool.tile([128, L + 1], fdt, tag="sq", bufs=3)
                    nc.sync.dma_start(sq[:], seq_view[:, lc, :])
                    st, sp = lc == 0, lc == 7
                    for rc in range(2):
                        lhsT = CTmm[:, lc, rc * 128:(rc + 1) * 128]
                        self.mm(ps_rel[rc][:, 0:512], lhsT, sq[:, 0:512],
                                start=st, stop=sp)
                        self.mm(ps_rel[rc][:, 512:1024], lhsT, sq[:, 512:1024],
                                start=st, stop=sp)
                        sl = (lhsT, sq[:, 1024:1025])
                        if self.mm_mode == "f32r":
                            sl = (sl[0].bitcast(F32), sl[1].bitcast(F32))
                        self.mm(ps_s8[:, rc, lc:lc + 1], sl[0], sl[1],
                                start=True, stop=True)
                for rc in range(2):
                    tdenom = gxpool.tile([128, 1], F32, tag="tden")
                    nc.vector.tensor_reduce(tdenom[:], ps_s8[:, rc, :],
                                            axis=mybir.AxisListType.X,
                                            op=mybir.AluOpType.add)
                    nc.scalar.activation(tdenom[:], tdenom[:], AF.Copy,
                                         bias=16e-5, scale=1.0)
                    frec = gxpool.tile([128, 1], F32, tag="frec")
                    nc.vector.reciprocal(frec[:], tdenom[:])
                    rel_sc = mid.tile([128, L], fdt, tag="rel_sc",
                                      name="rel_sc")
                    nc.vector.tensor_scalar_mul(rel_sc[:], ps_rel[rc][:],
                                                frec[:])
                    for dc in range(8):
                        ps_t = ps_sm.tile([128, 128], fdt, tag="sm")
                        self.tp(ps_t[:], rel_sc[:, dc * 128:(dc + 1) * 128],
                                ident[:])
                        nc.scalar.copy(relT[:, dc, rc * 128:(rc + 1) * 128],
                                       ps_t[:])

                # EW = ent_sb @ W[0:1024]: gather commutes with the linear
                # extractor half (computed here to overlap C/rel phases)
                entT = mid.tile([128, 8, E], fdt, name="entT")
                for hc in range(8):
                    ps_t2 = ps_sm.tile([128, E], fdt, tag="sm",
                                       name="ps_t2")
                    self.tp(ps_t2[:], ent_sb[:, hc * 128:(hc + 1) * 128],
                            ident[0:E, 0:E])
                    nc.scalar.copy(entT[:, hc, :], ps_t2[:])
                EWh = mid.tile([E, EMB], fdt, name="EWh")
                EWt = mid.tile([E, EMB], fdt, name="EWt")
                for w, ew in ((wh_sb, EWh), (wt_sb, EWt)):
                    ps_ew = ps_big.tile([E, EMB], F32, tag="big", name="ps_ew")
                    for hc in range(8):
                        for lo, hi in ((0, 512), (512, 768)):
                            self.mm(ps_ew[:, lo:hi], entT[:, hc, :],
                                    w[:, hc, lo:hi],
                                    start=(hc == 0), stop=(hc == 7))
                    nc.scalar.copy(ew[:], ps_ew[:])

            psR_cm.__exit__(None, None, None)
            # ---------- P5: extractors -> hsEt/tsEt [emb, n], per-ec -------
            # h/t interleaved per emb-chunk and staged to DRAM immediately so
            # classifier block k can start once chunk k//2 is staged.
            psE_cm = tc.tile_pool(name="psE", bufs=3, space="PSUM")
            ps_sm = psE_cm.__enter__()
            hsEt = late.tile([128, 6, RPC], fdt)
            tsEt = late.tile([128, 6, RPC], fdt)
            hsDs = [dramp.tile([128, RPC], fdt, name=f"hsD{e}")
                    for e in range(6)]
            tsDs = [dramp.tile([128, RPC], fdt, name=f"tsD{e}")
                    for e in range(6)]
            if True:
                for ec in range(6):
                    for (w, bvec, ew, oh, dst, dv) in (
                        (wh_sb, bh, EWh, ohx, hsEt, hsDs[ec]),
                        (wt_sb, bt, EWt, ohy, tsEt, tsDs[ec]),
                    ):
                        ps_e = ps_sm.tile([128, RPC], F32, tag="sm",
                                          name="ps_e")
                        self.mm(ps_e[:], ew[:, ec * 128:(ec + 1) * 128], oh[:],
                                start=True, stop=False)
                        for kc in range(8, 16):
                            self.mm(ps_e[:], w[:, kc, ec * 128:(ec + 1) * 128],
                                    relT[:, kc % 8, :],
                                    start=False, stop=(kc == 15))
                        nc.scalar.activation(dst[:, ec, :], ps_e[:], AF.Tanh,
                                             bias=bvec[:, ec:ec + 1])
                        nc.sync.dma_start(dv[:], dst[:, ec, :])
            psE_cm.__exit__(None, None, None)
            wpin_cm.__exit__(None, None, None)

            # ---------- P6: block bilinear + classifier ----------
            with (
                tc.tile_pool(name="blph", bufs=3) as blph,
                tc.tile_pool(name="ps_lt", bufs=1, space="PSUM") as ps_lt,
            ):
                pslt = ps_lt.tile([NCL, RPC], F32)
                for k in range(K12):
                    wb = blph.tile([128, 32 * NCL], fdt, tag="wb")
                    nc.sync.dma_start(
                        wb[:],
                        d["wb"].ap()[:, k * 32 * NCL:(k + 1) * 32 * NCL])
                    # b1rep[p, c, n] = hsEt_row(64k + 32*(p//64) + c)[n]
                    # (wb host layout matches: i = c + 32*(p//64), j = p%64)
                    kk = 64 * (k % 2)
                    # b2t[p, n] = tsEt_row(64k + p%64)[n]
                    b2t = blph.tile([128, RPC], fdt, tag="b2t")
                    for h0 in (0, 1):
                        nc.scalar.dma_start(b2t[64 * h0:64 * (h0 + 1)],
                                            tsDs[k // 2][kk:kk + 64, :])
                    blT = blph.tile([128, 32, RPC], fdt, tag="blT")
                    if k % 2 == 0:
                        # PE-route: replicate hsEt rows via one-hot matmuls,
                        # ACT-copy out of PSUM, multiply on DVE
                        hsE64 = hsEt[kk:kk + 64, k // 2, :]
                        for cq in range(8):
                            psR = ps_lt.tile([128, 4, RPC], F32, tag="rep",
                                             bufs=3, name="psR")
                            for i4 in range(4):
                                self.mm(psR[:, i4, :],
                                        repm[:, cq * 4 + i4, :], hsE64)
                            b1s = blph.tile([128, 4, RPC], fdt, tag="b1s",
                                            bufs=3, name="b1s")
                            nc.scalar.copy(b1s[:], psR[:])
                            b2b = b2t[:].unsqueeze(1).broadcast_to(
                                [128, 4, RPC])
                            nc.vector.tensor_mul(
                                blT[:, cq * 4:(cq + 1) * 4, :], b1s[:], b2b)
                    else:
                        b1rep = blph.tile([128, 32, RPC], fdt, tag="b1rep")
                        for h0 in (0, 1):
                            srcap = hsDs[k // 2] \
                                [kk + 32 * h0:kk + 32 * (h0 + 1), :] \
                                .unsqueeze(0).broadcast_to([64, 32, RPC])
                            nc.scalar.dma_start(b1rep[64 * h0:64 * (h0 + 1)],
                                                srcap)
                        for ch in range(4):
                            b2b = b2t[:].unsqueeze(1).broadcast_to(
                                [128, 8, RPC])
                            nc.vector.tensor_mul(
                                blT[:, 8 * ch:8 * (ch + 1), :],
                                b1rep[:, 8 * ch:8 * (ch + 1), :], b2b)
                    for c in range(32):
                        cg = k * 32 + c
                        self.mm(pslt[:], wb[:, c * NCL:(c + 1) * NCL],
                                blT[:, c, :],
                                start=(cg == 0), stop=(cg == 383))

                out_sb = late.tile([NCL, RPC], F32)
                nc.scalar.activation(out_sb[:], pslt[:], AF.Identity,
                                     bias=bbc[:])
                nc.sync.dma_start(d["lt"].ap(), out_sb[:])


_PROGRAM_CACHE = {}


def _get_program(mm_mode: str):
    if mm_mode not in _PROGRAM_CACHE:
        _PROGRAM_CACHE[mm_mode] = _Builder(mm_mode)
    return _PROGRAM_CACHE[mm_mode]


def _host_inputs(seq_lhs, ent_lhs, ent_to_seq_attn, entity_id_labels, hts,
                 Wh, bh, Wt, bt, Wb, bb):
    """Build the 8 per-core input maps (all host-side numpy)."""
    fdt = _np_fdt()
    seq_lhs = np.asarray(seq_lhs, np.float32)
    ent_lhs = np.asarray(ent_lhs, np.float32)
    ent_to_seq_attn = np.asarray(ent_to_seq_attn, np.float32)
    entity_id_labels = np.asarray(entity_id_labels)
    hts = np.asarray(hts)
    Wh = np.asarray(Wh, np.float32)
    Wt = np.asarray(Wt, np.float32)
    Wb = np.asarray(Wb, np.float32)
    bh = np.asarray(bh, np.float32)
    bt = np.asarray(bt, np.float32)
    bb = np.asarray(bb, np.float32)

    # device chunk (k, c) row p maps to Wb row k*4096 + i*64 + j with
    # i = c + 32*(p//64), j = p%64
    p_ = np.arange(128)
    c_ = np.arange(32)
    k_ = np.arange(K12)
    rows = (k_[:, None, None] * 4096
            + (c_[None, :, None] + 32 * (p_[None, None, :] // 64)) * 64
            + (p_[None, None, :] % 64))  # [k, c, p]
    wb_r = np.ascontiguousarray(
        Wb[rows.reshape(-1), :].reshape(K12 * 32, 128, NCL)
        .transpose(1, 0, 2).reshape(128, 384 * NCL)
    ).astype(fdt)
    wh_c = np.ascontiguousarray(
        Wh.reshape(16, 128, EMB).transpose(1, 0, 2).reshape(128, 16 * EMB)
    ).astype(fdt)
    wt_c = np.ascontiguousarray(
        Wt.reshape(16, 128, EMB).transpose(1, 0, 2).reshape(128, 16 * EMB)
    ).astype(fdt)
    bh_c = np.ascontiguousarray(bh.reshape(EMB // 128, 128).T)
    bt_c = np.ascontiguousarray(bt.reshape(EMB // 128, 128).T)
    bb_c = np.ascontiguousarray(bb.reshape(NCL, 1))
    ident = np.eye(128, dtype=np.float32).astype(fdt)
    # repm[r, c, p] = 1 iff r == c + 32*(p//64)
    repm_h = np.zeros((E, 32, 128), np.float32)
    for c in range(32):
        repm_h[c, c, 0:64] = 1.0
        repm_h[c + 32, c, 64:128] = 1.0
    repm_h = repm_h.reshape(E, 32 * 128).astype(fdt)

    in_maps = []
    for c in range(NCORES):
        doc, half = divmod(c, 2)
        sl = slice(half * RPC, (half + 1) * RPC)
        labels = entity_id_labels[doc].astype(np.int64)
        cnt = np.bincount(labels, minlength=E).astype(np.float32)
        S = np.zeros((M, E), np.float32)
        S[np.arange(M), labels] = 1.0
        smean = S / np.maximum(cnt, 1.0)[None, :]  # [M, E]
        eadd = (cnt == 0).astype(np.float32).reshape(E, 1)
        hi = hts[doc, sl, 0].astype(np.int64)
        ti = hts[doc, sl, 1].astype(np.int64)
        ohx = np.zeros((E, RPC), np.float32)
        ohx[hi, np.arange(RPC)] = 1.0
        ohy = np.zeros((E, RPC), np.float32)
        ohy[ti, np.arange(RPC)] = 1.0
        ohxy2 = np.concatenate([smean @ ohx, smean @ ohy], axis=1)  # [M, 512]
        attn = np.ascontiguousarray(
            ent_to_seq_attn[doc].transpose(1, 0, 2).reshape(M, NH * L)
        ).astype(fdt)
        seq_r = seq_lhs[doc].reshape(8, 128, L).transpose(1, 0, 2)
        seq_aug = np.concatenate(
            [seq_r, np.ones((128, 8, 1), np.float32)], axis=2
        )
        in_maps.append({
            "ent": np.ascontiguousarray(ent_lhs[doc]),
            "attn": attn,
            "seq": np.ascontiguousarray(seq_aug.reshape(128, 8 * (L + 1))).astype(fdt),
            "ssum": S.astype(fdt),
            "ohxy2": ohxy2.astype(fdt),
            "eadd": eadd,
            "ohx": ohx.astype(fdt),
            "ohy": ohy.astype(fdt),
            "wh": wh_c, "wt": wt_c, "bh": bh_c, "bt": bt_c,
            "wb": wb_r, "bbc": bb_c, "ident": ident, "repm": repm_h,
        })
    return in_maps


_LAST_RESULTS = {}


def kernel(**inputs) -> np.ndarray:
    prog = _get_program(MM_MODE)
    in_maps = _host_inputs(**inputs)
    trace = os.environ.get("DOCRED_TRACE", "0") == "1"
    res = run_bass_kernel_spmd(
        prog.nc, in_maps, core_ids=list(range(NCORES)), trace=trace,
    )
    _LAST_RESULTS["res"] = res
    out = np.empty((B * R, NCL), np.float32)
    for c in range(NCORES):
        doc, half = divmod(c, 2)
        lt = res.results[c]["lt"]  # [NCL, RPC]
        out[doc * R + half * RPC: doc * R + (half + 1) * RPC, :] = lt.T
    return out
